# revision 22
# baseline (speedup 1.0000x reference)
"""Trainium2 Bass kernel for the DSConv1d block (relu -> BN(eval) -> depthwise
conv1d(k=3,pad=1) -> PReLU -> GlobalLayerNorm -> pointwise conv -> residual).

Sharding: data-parallel over batch B=16 across 8 NeuronCores (2 samples/core).
Everything per-sample is device-local; no collectives.

v3 design (vs the fp32r baseline at ~189us):
  * bf16 I/O: x uploads and y stores as bf16 (host casts), halving HBM
    traffic. The phase-2 pointwise GEMM runs on bf16 operands (pt, wsc) --
    same 1 cycle/column PE rate as fp32r, all accesses 4B-aligned.
  * The conv halo tile g stays float32r (as in the baseline): the +-1
    element shifts of the depthwise taps keep 4-byte alignment, which DVE
    needs for full throughput (bf16 odd-element offsets run ~7x slower).
  * The two samples are software-pipelined in program order so the PE
    stream never waits on the per-sample gLN stats barrier; sample 0's
    conv runs fully on the PE (fast head), sample 1's conv is partially
    offloaded to an ACT/DVE tap chain under sample 0's pointwise GEMMs.
  * GPSIMD only does partition broadcasts: its tensor ops measure ~11x
    slower than roofline on this hardware.
"""

import numpy as np

B, C, T = 16, 512, 4000
NCORES = 8
BPC = B // NCORES          # samples per core
CT = 4                     # channel tiles of 128
P = 128
TH = 2                     # halves of T
HW_ = T // TH              # 2000
CHUNKS = [(0, 512), (512, 512), (1024, 512), (1536, 464)]  # bank-aligned
BN_EPS = 1e-5
GLN_EPS = 1e-8

# Per-sample group index idx = ci*2 + h (0..7).
OFF_PE = {0: set(), 1: set()}            # taps via ACT/DVE instead of PE
RELU_ACT = {0: set(), 1: set()}          # relu on ACT (rest on DVE)
SQ_DVE = {0: {0, 2, 4, 6}, 1: {0, 2, 4, 6}}  # square-sum on DVE
EPI_ACT = {0: set(), 1: {0, 1, 2, 3, 4, 5}}  # epi = ACT(+d), DVE(+x)

_CACHE = {}


def _build(alpha: float):
    import concourse.bass as bass
    import concourse.mybir as mybir
    import concourse.tile as tile
    from concourse import bacc

    f32 = mybir.dt.float32
    f32r = mybir.dt.float32r
    bf16 = mybir.dt.bfloat16
    AF = mybir.ActivationFunctionType
    OP = mybir.AluOpType
    AX = mybir.AxisListType

    nc = bacc.Bacc("TRN2", target_bir_lowering=False, debug=False)

    x_d = nc.dram_tensor("x", [BPC, C, T], bf16, kind="ExternalInput")
    dg_d = nc.dram_tensor("dg", [P, CT * 3 * P], f32, kind="ExternalInput")
    cv_d = nc.dram_tensor("cv", [P, CT * 7], f32, kind="ExternalInput")
    wt_d = nc.dram_tensor("wt", [P, CT * C], f32, kind="ExternalInput")
    y_d = nc.dram_tensor("y", [BPC, C, T], bf16, kind="ExternalOutput")

    with tile.TileContext(nc) as tc:
        with (
            tc.tile_pool(name="cpool", bufs=1) as cpool,
            tc.tile_pool(name="xpool", bufs=2 * CT) as xpool,
            tc.tile_pool(name="ppool", bufs=2 * CT) as ppool,
            tc.tile_pool(name="gpool", bufs=2) as gpool,
            tc.tile_pool(name="scp", bufs=2) as scp,
            tc.tile_pool(name="jpool", bufs=3) as jpool,
            tc.tile_pool(name="opool", bufs=2) as opool,
            tc.tile_pool(name="wscp", bufs=8) as wscp,
            tc.tile_pool(name="spool", bufs=2) as spool,
            tc.tile_pool(name="pspool", bufs=2, space=bass.MemorySpace.PSUM) as pspool,
        ):
            # ---- constants first (small; the first taps need dg), then x ----
            dgstg = cpool.tile([P, CT * 3 * P], f32, tag="dgstage")
            nc.sync.dma_start(dgstg[:], dg_d[:])
            cblk = cpool.tile([P, CT * 7], f32, tag="cblk")
            nc.sync.dma_start(cblk[:], cv_d[:])

            xt = {}       # (b, ci) -> tile
            pt = {}       # (b, ci) -> tile
            for b in range(BPC):
                for ci in range(CT):
                    x_ = xpool.tile([P, T], bf16, tag="x", name=f"x{b}_{ci}")
                    nc.sync.dma_start(x_[:], x_d[b, ci * P:(ci + 1) * P, :])
                    xt[(b, ci)] = x_
                    pt[(b, ci)] = ppool.tile([P, T], bf16, tag="p",
                                             name=f"pt{b}_{ci}")

            wtall = cpool.tile([P, CT * C], f32, tag="wtall")
            nc.sync.dma_start(wtall[:], wt_d[:])
            warm = cpool.tile([P, 1], f32, tag="warm")
            nc.scalar.activation(warm[:], cblk[:, 0:1], AF.Identity)
            def wv(c):
                return cblk[:, c:c + 1]

            def pads(c):
                return cblk[:, CT * 3 + c:CT * 3 + c + 1]

            def bsum(c):
                return cblk[:, CT * 4 + c:CT * 4 + c + 1]

            wgam = cblk[:, CT * 5:CT * 6]
            wbet = cblk[:, CT * 6:CT * 7]

            diag_sb = {}

            def cast_dg(ci):
                row = []
                for k in range(3):
                    t = cpool.tile([P, P], f32r, tag=f"dg{ci}{k}",
                                   name=f"dgr{ci}{k}")
                    if ci == 0:
                        nc.scalar.activation(
                            t[:], dgstg[:, (ci * 3 + k) * P:(ci * 3 + k + 1) * P],
                            AF.Identity)
                    else:
                        nc.vector.tensor_copy(
                            t[:], dgstg[:, (ci * 3 + k) * P:(ci * 3 + k + 1) * P])
                    row.append(t)
                diag_sb[ci] = row

            wt_sb = [wtall[:, k * C:(k + 1) * C] for k in range(CT)]
            ones = cpool.tile([P, 1], f32, tag="ones")
            nc.vector.memset(ones[:], 1.0)
            sums = {}     # b -> [P, 16]
            wsc = {}      # (b, k) -> tile
            dsh = {}      # b -> [P, CT]

            def ph1_group(b, ci, h):
                idx = ci * 2 + h
                o0 = h * HW_
                x_ = xt[(b, ci)]
                g = gpool.tile([P, HW_ + 2], f32r, tag="g")
                if idx in RELU_ACT[b]:
                    nc.scalar.activation(g[:, 1:HW_ + 1], x_[:, o0:o0 + HW_],
                                         AF.Relu)
                else:
                    nc.vector.tensor_scalar_max(
                        g[:, 1:HW_ + 1], x_[:, o0:o0 + HW_], 0.0)
                if h == 0:
                    nc.vector.tensor_copy(g[:, 0:1], pads(ci))
                else:
                    nc.vector.tensor_scalar_max(
                        g[:, 0:1], x_[:, o0 - 1:o0], 0.0)
                if h == TH - 1:
                    nc.vector.tensor_copy(g[:, HW_ + 1:HW_ + 2], pads(ci))
                else:
                    nc.vector.tensor_scalar_max(
                        g[:, HW_ + 1:HW_ + 2], x_[:, o0 + HW_:o0 + HW_ + 1],
                        0.0)

                pslice = pt[(b, ci)][:, o0:o0 + HW_]
                if idx not in OFF_PE[b]:
                    cps = pspool.tile([P, 2048], f32, tag="ps")
                    for k in range(3):
                        for c0, wc in CHUNKS:
                            nc.tensor.matmul(
                                cps[:, c0:c0 + wc],
                                diag_sb[ci][k][:],
                                g[:, k + c0: k + c0 + wc],
                                start=(k == 0), stop=(k == 2))
                    nc.scalar.activation(
                        pslice, cps[:, 0:HW_], AF.Prelu,
                        bias=bsum(ci), scale=1.0, alpha=alpha,
                        accum_out=sums[b][:, idx:idx + 1])
                else:
                    t1 = scp.tile([P, HW_], f32, tag="sc", name=f"t1_{b}{idx}")
                    t2 = scp.tile([P, HW_], f32, tag="sc", name=f"t2_{b}{idx}")
                    t3 = scp.tile([P, HW_], f32, tag="sc", name=f"t3_{b}{idx}")
                    nc.scalar.activation(
                        t1[:], g[:, 0:HW_], AF.Identity,
                        bias=bsum(ci), scale=wv(ci * 3))
                    nc.vector.scalar_tensor_tensor(
                        t2[:], g[:, 1:HW_ + 1], wv(ci * 3 + 1),
                        t1[:], OP.mult, OP.add)
                    nc.vector.scalar_tensor_tensor(
                        t3[:], g[:, 2:HW_ + 2], wv(ci * 3 + 2),
                        t2[:], OP.mult, OP.add)
                    nc.scalar.activation(
                        pslice, t3[:], AF.Prelu, bias=0.0, scale=1.0,
                        alpha=alpha, accum_out=sums[b][:, idx:idx + 1])

            def sq_group(b, ci, h):
                idx = ci * 2 + h
                o0 = h * HW_
                pslice = pt[(b, ci)][:, o0:o0 + HW_]
                acc = sums[b][:, 8 + idx:9 + idx]
                junk = jpool.tile([P, HW_], bf16, tag="jk", name=f"jk_{b}{idx}")
                if idx in SQ_DVE[b]:
                    nc.vector.scalar_tensor_tensor(
                        junk[:], pslice, 1.0, pslice, OP.mult, OP.mult,
                        accum_out=acc)
                else:
                    nc.scalar.activation(junk[:], pslice, AF.Square,
                                         accum_out=acc)

            def stats(b):
                spr = pspool.tile([1, 16], f32, tag="ps")
                nc.tensor.matmul(spr[0:1, :], ones[:], sums[b][:], start=True,
                                 stop=True)
                st = spool.tile([1, 16], f32, tag="st")
                iS, iQ, iMEAN, iE2, iMSQ, iVAR, iA, iS0, iR0, iAR, iS1, \
                    iRSTD, iRM = range(13)

                def stc(i):
                    return st[0:1, i:i + 1]

                nc.vector.tensor_reduce(stc(iS), spr[0:1, 0:8], AX.X, OP.add)
                nc.vector.tensor_reduce(stc(iQ), spr[0:1, 8:16], AX.X, OP.add)
                invN = 1.0 / float(C * T)
                nc.vector.tensor_scalar_mul(stc(iMEAN), stc(iS), invN)
                nc.vector.tensor_scalar_mul(stc(iE2), stc(iQ), invN)
                nc.vector.tensor_scalar(stc(iMSQ), stc(iMEAN), stc(iMEAN),
                                        None, OP.mult)
                nc.vector.scalar_tensor_tensor(stc(iVAR), stc(iMSQ), -1.0,
                                               stc(iE2), OP.mult, OP.add)
                nc.vector.tensor_scalar_add(stc(iA), stc(iVAR), GLN_EPS)
                nc.scalar.activation(stc(iS0), stc(iA), AF.Sqrt)
                nc.vector.reciprocal(stc(iRSTD), stc(iS0))
                nc.vector.tensor_scalar(stc(iRM), stc(iRSTD), stc(iMEAN),
                                        -1.0, OP.mult, OP.mult)
                rstd_b = spool.tile([P, 1], f32, tag="rstd_b")
                rm_b = spool.tile([P, 1], f32, tag="rm_b")
                nc.gpsimd.partition_broadcast(rstd_b[:], stc(iRSTD))
                nc.gpsimd.partition_broadcast(rm_b[:], stc(iRM))
                d = spool.tile([P, CT], f32, tag="d")
                nc.vector.scalar_tensor_tensor(d[:], wgam[:], rm_b[:, 0:1],
                                               wbet[:], OP.mult, OP.add)
                dsh[b] = d
                for k in range(CT):
                    t = wscp.tile([P, C], bf16, tag="wsc",
                                  name=f"wsc_{b}{k}")
                    nc.scalar.activation(t[:], wt_sb[k], AF.Identity,
                                         bias=0.0, scale=rstd_b[:, 0:1])
                    wsc[(b, k)] = t

            def ph2_group(b, oi, h, split=False):
                o0 = h * HW_
                ops = pspool.tile([P, 2048], f32, tag="ps")
                parts = ([CHUNKS[:2], CHUNKS[2:]] if split else [CHUNKS])
                ot = opool.tile([P, HW_], bf16, tag="o")
                lo = 0
                for part in parts:
                    for k in range(CT):
                        for c0, wc in part:
                            nc.tensor.matmul(
                                ops[:, c0:c0 + wc],
                                wsc[(b, k)][:, oi * P:(oi + 1) * P],
                                pt[(b, k)][:, o0 + c0: o0 + c0 + wc],
                                start=(k == 0), stop=(k == CT - 1))
                    hi = min(part[-1][0] + part[-1][1], HW_)
                    if oi * 2 + h in EPI_ACT[b]:
                        tm = jpool.tile([P, HW_], bf16, tag="jk",
                                        name=f"ep_{b}{oi}{h}{lo}")
                        nc.scalar.activation(
                            tm[:, lo:hi], ops[:, lo:hi], AF.Identity,
                            bias=dsh[b][:, oi:oi + 1], scale=1.0)
                        nc.vector.tensor_tensor(
                            ot[:, lo:hi], tm[:, lo:hi],
                            xt[(b, oi)][:, o0 + lo:o0 + hi], OP.add)
                    else:
                        nc.vector.scalar_tensor_tensor(
                            ot[:, lo:hi], ops[:, lo:hi],
                            dsh[b][:, oi:oi + 1],
                            xt[(b, oi)][:, o0 + lo:o0 + hi], OP.add, OP.add)
                    nc.sync.dma_start(
                        y_d[b, oi * P:(oi + 1) * P, o0 + lo:o0 + hi],
                        ot[:, lo:hi])
                    lo = hi

            def ph1_prefix(b, cis, with_sq=False):
                for ci in cis:
                    for h in range(TH):
                        ph1_group(b, ci, h)
                        if with_sq:
                            sq_group(b, ci, h)

            # ---------------- program order ----------------
            sums[0] = spool.tile([P, 16], f32, tag="sums", name="sums0")
            sums[1] = spool.tile([P, 16], f32, tag="sums", name="sums1")

            for ci in range(CT):
                cast_dg(ci)
            ph1_prefix(0, range(CT), with_sq=True)      # ph1(b0) + inline sq
            ph1_prefix(1, [0], with_sq=True)            # ph1(b1) g0,g1
            stats(0)
            for ci in (1, 2, 3):                        # interleave b1 conv
                ph1_prefix(1, [ci], with_sq=True)       # with b0 pointwise
                for h in range(TH):
                    ph2_group(0, ci - 1, h)
            ph2_group(0, 3, 0)
            stats(1)
            ph2_group(0, 3, 1)
            for oi in range(CT):                        # ph2(b1)
                for h in range(TH):
                    ph2_group(1, oi, h, split=(oi == CT - 1 and h == TH - 1))

    nc.compile()
    return nc


def _host_prep(bn_gamma, bn_beta, bn_mean, bn_var, dw_w, gln_gamma, gln_beta,
               pw_w):
    f64 = np.float64
    s = bn_gamma.astype(f64) / np.sqrt(bn_var.astype(f64) + BN_EPS)
    bb = bn_beta.astype(f64) - bn_mean.astype(f64) * s
    w = dw_w[:, 0, :].astype(f64)                      # [C, 3]
    sw = s[:, None] * w                                # [C, 3]
    dg = np.zeros((CT * 3, P, P), np.float32)
    for ci in range(CT):
        sl = slice(ci * P, (ci + 1) * P)
        for k in range(3):
            dg[ci * 3 + k] = np.diag(sw[sl, k]).astype(np.float32)
    wv = sw.reshape(CT, P, 3).transpose(1, 0, 2).reshape(P, CT * 3) \
        .astype(np.float32)
    s_safe = np.where(np.abs(s) < 1e-12, 1e-12, s)
    pads = (-bb / s_safe).reshape(CT, P).T.astype(np.float32)        # [P,CT]
    bsum = (bb * w.sum(1)).reshape(CT, P).T.astype(np.float32)
    wtT = (pw_w.astype(f64) * gln_gamma.astype(f64)[None, :]).T      # [C, O]
    wt = np.ascontiguousarray(
        wtT.reshape(CT, P, C).transpose(1, 0, 2).reshape(P, CT * C)
        .astype(np.float32))
    wgam = (pw_w.astype(f64) @ gln_gamma.astype(f64)).reshape(CT, P).T \
        .astype(np.float32)
    wbet = (pw_w.astype(f64) @ gln_beta.astype(f64)).reshape(CT, P).T \
        .astype(np.float32)
    dgp = np.ascontiguousarray(
        dg.reshape(CT * 3, P, P).transpose(1, 0, 2).reshape(P, CT * 3 * P))
    cv = np.concatenate([wv, pads, bsum, wgam, wbet], axis=1)
    return dict(dg=dgp, cv=np.ascontiguousarray(cv), wt=wt)


def _get_program(alpha: float):
    key = round(float(alpha), 9)
    if key not in _CACHE:
        _CACHE[key] = _build(float(alpha))
    return _CACHE[key]


def run(inputs: dict, trace: bool = False):
    """Run on 8 cores; returns (y_full, BassKernelResults)."""
    import ml_dtypes
    from concourse.bass_utils import run_bass_kernel_spmd

    inputs = {k: np.asarray(v) for k, v in inputs.items()}
    x = np.ascontiguousarray(inputs["x"]).astype(ml_dtypes.bfloat16)
    alpha = float(np.asarray(inputs["prelu_a"]).reshape(-1)[0])
    consts = _host_prep(
        inputs["bn_gamma"], inputs["bn_beta"], inputs["bn_mean"],
        inputs["bn_var"], inputs["dw_w"], inputs["gln_gamma"],
        inputs["gln_beta"], inputs["pw_w"])
    nc = _get_program(alpha)
    in_maps = [
        {"x": x[i * BPC:(i + 1) * BPC], **consts} for i in range(NCORES)
    ]
    res = run_bass_kernel_spmd(nc, in_maps, list(range(NCORES)), trace=trace)
    y = np.concatenate(
        [res.results[i]["y"].astype(np.float32) for i in range(NCORES)],
        axis=0)
    return y, res


def kernel(**inputs) -> np.ndarray:
    y, _ = run(inputs)
    return y
